# revision 1
# baseline (speedup 1.0000x reference)
"""CARAFE upsampling (k=5, x2, C=256) as a Bass/Tile kernel on 8 NeuronCores.

Math (per output pixel):
  out[b, Y, X, c] = sum_{ky,kx} softmax(masks[b,Y,X,:])[ky*5+kx]
                    * feat[b, Y//2+ky-2, X//2+kx-2, c]       (zero padded)

Mapping: pure data parallel over (batch, 32-output-row strips) -> 8 cores,
no collectives. Each core handles 16 output-row pairs (rows Y=2j, 2j+1
share source rows). Host-side zero padding (2 extra rows AND columns on
each side of the feature slab) makes all cores run an identical
edge-case-free program.

Per row pair, X is split into 4 blocks of 32. For block b the contraction
is K = (ky in 5) x (t in 20) = 100 over padded source column xp = 16b + t:
  out[X=32b+X_loc, c] = sum_K lhsT[(ky,t), X_loc] * wnd[(ky,t), b*256+c]
  lhsT[(ky,t), X_loc] = exp(masks[Y, X, ky*5 + (t - X_loc//2)])  (band, 0)
One matmul per (row r, block b): M=32, K=100, N=256, issued to 4 distinct
column groups via tile_position=(0, 32b) so the 4 blocks execute
concurrently in the PE array (fp32 measured ~90 ns/mm vs ~394 sequential).

The banded lhsT needs a diagonal (partition+byte coupled) scatter that the
SBUF descriptor generators cannot express, so the scatter writes a flat
DRAM staging tile (arbitrary affine strides are legal there), one DMA per
ky plane, and a plain 2D load brings the packed [100, 256] lhsT back to
SBUF. Out-of-band staging bytes are zeroed once and never dirtied (the
in-band positions are rewritten every iteration).

The feature window wnd[100, 4*256] is a ring buffer over ky slots: slab
row s always lives in slot s%5, so each row pair loads only one new row
slice (a single DMA thanks to column padding); the weight scatter writes
ky planes at kyp=(j+ky)%5 to match.

Softmax: numerators exp() are scattered unnormalized; denominators come
from a second natural-layout mask load + exp with accum_out (per-partition
sums land directly in [X=128,1] layout), reciprocal, and a fused
per-partition scale on the PSUM->SBUF eviction.

DMA instruction count is the scarce resource (each dma_start costs
~0.6 us of issuing-sequencer time), so mask loads are batched 4 pairs at
a time, outputs 2 rows per DMA, and issues are spread over the SP, Pool
and DVE sequencers while ACT only runs the exps.
"""

import sys

for _p in ("/opt/trn_rl_repo",):
    if _p not in sys.path:
        sys.path.insert(0, _p)

import numpy as np

B = 2
H_IN = 64
W_IN = 64
C = 256
H_OUT = 128
W_OUT = 128
KK = 25
N_CORES = 8
ROWS_PER_CORE = H_OUT * B // N_CORES  # 32 output rows
PAIRS = ROWS_PER_CORE // 2  # 16
SLAB = PAIRS + 4  # feature rows a core touches (16 + 2 pad each side)
WPAD = W_IN + 4  # 68 padded feature columns
NBLK = 4  # X blocks per row
UB = 16  # u (column pairs) per block
TW = 20  # t window width per block
KDIM = 5 * TW  # matmul contraction size
QUAD = 4  # pairs per mask-load batch

_NC_CACHE = {}


def _build_nc(reps=1):
    import concourse.bacc as bacc
    import concourse.mybir as mybir
    from concourse import tile

    dt = mybir.dt
    f32 = dt.float32

    nc = bacc.Bacc("TRN2", target_bir_lowering=False, debug=False,
                   num_devices=N_CORES)
    feat = nc.dram_tensor("feat", [SLAB, WPAD, C], f32, kind="ExternalInput")
    masks = nc.dram_tensor("masks", [ROWS_PER_CORE, W_OUT, KK], f32,
                           kind="ExternalInput")
    out = nc.dram_tensor("out", [ROWS_PER_CORE, W_OUT, C], f32,
                         kind="ExternalOutput")

    AP = type(feat[:])
    NBUF = 3

    with tile.TileContext(nc) as tc:
        with (
            tc.tile_pool(name="big", bufs=1) as big,
            tc.tile_pool(name="psum", bufs=4, space="PSUM") as psumpool,
        ):
            def mk(shape, tag, n):
                return [big.tile(shape, f32, tag=f"{tag}{i}",
                                 name=f"{tag}{i}") for i in range(n)]

            wnds = mk([KDIM, NBLK * C], "wnd", 2)
            lws = mk([KDIM, 2 * W_OUT], "lw", 2)
            m5s = mk([16, NBLK * 2 * QUAD * 50], "m5", 2)  # [16, 1600]
            m2s = mk([128, 2 * QUAD * KK], "m2", 2)  # [128, 200]
            e5s = mk([16, 400], "e5", NBUF)
            escr = mk([128, KK], "escr", 2)
            esums = mk([128, 2 * QUAD], "esum", 2)
            invsums = mk([128, 2 * QUAD], "invsum", 2)
            ots = mk([128, 2 * C], "ot", 3)

            stage = nc.dram_tensor("lw_stage", [2, KDIM, 2 * W_OUT], f32,
                                   kind="Internal")

            for w in wnds:
                nc.vector.memset(w[:], 0.0)
            nc.vector.memset(lws[0][:], 0.0)
            for hb in range(2):
                nc.sync.dma_start(out=stage[hb], in_=lws[0][:])

            def load_wnd_row(w, s, eng):
                """Load slab feature row s into ky slot s%5 of window w."""
                if s >= SLAB:
                    return
                slot = s % 5
                eng.dma_start(
                    out=w[slot * TW:(slot + 1) * TW, :].rearrange(
                        "t (b c) -> t b c", b=NBLK),
                    # overlapping block windows: src col = 16b + t (padded)
                    in_=AP(tensor=feat[:].tensor,
                           offset=feat[:].offset + s * WPAD * C,
                           ap=[[C, TW], [UB * C, NBLK], [1, C]]),
                )

            # prologue: window A holds rows 0-4 (pair 0), B rows 1-5 (pair 1)
            for s in range(5):
                load_wnd_row(wnds[0], s, nc.sync)
            for s in range(1, 6):
                load_wnd_row(wnds[1], s, nc.scalar)

            for j in range(PAIRS * reps):
                rep, j = divmod(j, PAIRS)
                hb = j % 2
                q, jq = divmod(j, QUAD)  # quad index, pair within quad
                lw = lws[hb]
                wnd = wnds[hb]

                if jq == 0:
                    # masks for 8 rows (4 pairs), u_loc on partitions:
                    # m5[u_loc, b*400 + r8*50 + v*25 + p]
                    m5 = m5s[q % 2]
                    for b in range(NBLK):
                        eng = [nc.sync, nc.scalar, nc.scalar, nc.sync][b]
                        eng.dma_start(
                            out=m5[:, b * 400:(b + 1) * 400].rearrange(
                                "u (r v p) -> u r (v p)", r=2 * QUAD, v=2),
                            in_=masks[8 * q:8 * q + 8, 32 * b:32 * (b + 1)]
                            .rearrange("r (u v) p -> u r (v p)", v=2),
                        )
                    # natural layout for softmax denominators
                    m2 = m2s[q % 2]
                    nc.sync.dma_start(
                        out=m2[:].rearrange("x (r p) -> x r p", r=2 * QUAD),
                        in_=masks[8 * q:8 * q + 8].rearrange(
                            "r x p -> x r p"),
                    )
                    esum = esums[q % 2]
                    for r in range(2 * QUAD):
                        nc.scalar.activation(
                            out=escr[r % 2][:],
                            in_=m2[:, r * KK:(r + 1) * KK],
                            func=mybir.ActivationFunctionType.Exp,
                            accum_out=esum[:, r:r + 1],
                        )
                    invsum = invsums[q % 2]
                    nc.vector.reciprocal(invsum[:], esum[:])
                m5 = m5s[q % 2]
                invsum = invsums[q % 2]

                # exp into scatter-ready layout:
                # e5[u_loc, ky*80 + kx*16 + b*4 + v*2 + r]
                e5 = e5s[j % NBUF]
                m5v = m5[:].rearrange(
                    "u (b r v ky kx) -> u b r v ky kx",
                    b=NBLK, r=2 * QUAD, v=2, ky=5)
                e5v = e5[:].rearrange("u (ky kx b v r) -> u b r v ky kx",
                                      ky=5, kx=5, b=NBLK, v=2)
                for r in range(2):
                    for b in range(NBLK):
                        nc.scalar.activation(
                            out=e5v[:, b, r],
                            in_=m5v[:, b, 2 * jq + r],
                            func=mybir.ActivationFunctionType.Exp,
                        )

                # Banded scatter via DRAM staging (flat -> diagonal legal):
                # stage[hb][kyp*20 + u_loc + kx, 64b + 4u_loc + 2v + r]
                #   = e5[u_loc, ky*80 + kx*16 + b*4 + v*2 + r]
                e5_full = e5[:]
                st_full = stage[hb]
                sceng = [nc.sync, nc.scalar, nc.scalar, nc.sync, nc.sync]
                for ky in range(5):
                    kyp = (j + ky) % 5
                    dst = AP(tensor=st_full.tensor,
                             offset=st_full.offset + kyp * TW * 2 * W_OUT,
                             ap=[[260, 16], [64, 20], [1, 4]])
                    src = AP(tensor=e5_full.tensor,
                             offset=e5_full.offset + ky * 80,
                             ap=[[400, 16], [4, 20], [1, 4]])
                    sceng[ky].dma_start(out=dst, in_=src)
                # load the packed lhsT back into SBUF
                nc.scalar.dma_start(out=lw[:], in_=stage[hb])

                # one matmul per (row, block); blocks run concurrently in
                # distinct PE column groups
                lwv = lw[:].rearrange("k (x v r) -> k x v r", v=2, r=2)
                ot = ots[j % 3]
                for r in range(2):
                    ps = psumpool.tile([128, C], f32, tag="ps", name="ps")
                    for b in range(NBLK):
                        lhsT = lwv[:, UB * b:UB * (b + 1), :, r]
                        nc.tensor.matmul(ps[32 * b:32 * (b + 1), :],
                                         lhsT,
                                         wnd[:, b * C:(b + 1) * C],
                                         start=True, stop=True,
                                         tile_position=(0, 32 * b))
                    nc.vector.tensor_scalar_mul(
                        ot[:, r * C:(r + 1) * C], ps[:],
                        invsum[:, 2 * jq + r:2 * jq + r + 1])
                # both output rows in one DMA: out[2j+r, x, c]
                nc.sync.dma_start(
                    out=out[2 * j:2 * j + 2].rearrange("r x c -> x r c"),
                    in_=ot[:].rearrange("x (r c) -> x r c", r=2),
                )
                # prefetch rows j+5, j+6 for pair j+2 (emitted AFTER this
                # pair's matmuls so the WAR ordering is correct; the 2-pair
                # buffer slack keeps the load off the critical path)
                load_wnd_row(wnd, j + 5, nc.sync)
                load_wnd_row(wnd, j + 6, nc.scalar)

    nc.compile()
    return nc


def get_nc(reps=1):
    key = reps
    if key not in _NC_CACHE:
        _NC_CACHE[key] = _build_nc(reps)
    return _NC_CACHE[key]


def shard_inputs(features, masks):
    """Full inputs -> per-core input maps (host-side zero padding of both
    rows and columns of the feature slab)."""
    features = np.asarray(features)
    masks = np.asarray(masks)
    in_maps = []
    for c in range(N_CORES):
        b, q = divmod(c, 4)
        y0 = PAIRS * q
        slab = np.zeros((SLAB, WPAD, C), np.float32)
        lo = y0 - 2
        for i in range(SLAB):
            y = lo + i
            if 0 <= y < H_IN:
                slab[i, 2:2 + W_IN] = features[b, y]
        in_maps.append({
            "feat": np.ascontiguousarray(slab),
            "masks": np.ascontiguousarray(
                masks[b, ROWS_PER_CORE * q:ROWS_PER_CORE * (q + 1)]
            ).astype(np.float32),
        })
    return in_maps


def unshard_outputs(results):
    out = np.empty((B, H_OUT, W_OUT, C), np.float32)
    for c in range(N_CORES):
        b, q = divmod(c, 4)
        out[b, ROWS_PER_CORE * q:ROWS_PER_CORE * (q + 1)] = results[c]["out"]
    return out


def kernel(features, masks):
    from concourse.bass_utils import run_bass_kernel_spmd

    nc = get_nc()
    in_maps = shard_inputs(features, masks)
    res = run_bass_kernel_spmd(nc, in_maps, list(range(N_CORES)))
    return unshard_outputs(res.results)

